# revision 31
# baseline (speedup 1.0000x reference)
"""Trainium2 Bass kernel for nn_PitchRegisterTracker.

Algorithm notes
---------------
The reference maintains a size-1000 circular buffer of log2-pitches of the
valid (>0) frames, then normalizes every valid frame by the buffer's
mean/unbiased-std.  Because slot j keeps the *highest-rank* writer, the full
buffer is exactly the last min(1000, n_valid) valid elements.  So:

  phase A: stats (mean/std of ln-pitch) over the last 1000 valid elements.
           Only a small tail window of the input can contain them; every core
           redundantly computes the same stats from the same tail (no
           collectives needed).
  phase B: fully data-parallel elementwise map
           out = exp(sc * ln(p) + bi),  out(0) = 0 via exp(ln(0)) = exp(-inf).

where, with ln-space stats meanL/stdL over the selected values:
  sc = TLS*ln2/stdL   bi = ln2*TLM - meanL*sc
matching the reference's exp2((log2p - mean2)/std2 * TLS + TLM).

Performance notes (all HW-profiled on this part)
------------------------------------------------
The kernel is HBM-bandwidth bound (16 MiB in + 16 MiB out per core).  HWDGE
splits a DMA's descriptors over d SDMA engines, d = largest divisor of the
outermost AP dim <= 16, in contiguous chunks from engine 0.  [128]-outer
DMAs run ~27 GB/s/engine at 13-41 KB descriptors.  Quirks built around:
 - SDMA engine 15 degrades ~25% on some runs (random per run).  Insurance:
   per-core data is split into region 1 [128 x R1] moved as [128, w] DMAs
   (all 16 engines) and region 2 [120 x R2] moved as [120, w] DMAs
   (engines 0-14 only), so engine 15 sees ~0.82x the bytes of the others.
 - [120]-outer LOADS run ~13.5 GB/s with >=7 KB descriptors but ~24 GB/s
   with ~3-5 KB ones -> region-2 loads go in 1280-column chunks.
   [120]-outer STORES are full rate up to 14336-byte descriptors.
 - HWDGE descriptor generation is per-ring (~45ns/desc); one ring cannot
   sustain loads+stores.  Loads ride SP's ring (phase-A inputs first);
   every store rides ACT's ring inline right after its slice's Exp.
 - ACT (Ln+Exp, ~60us busy) paces store availability: tiles are loaded
   smallest-first so ACT starts early, and activated in ~3-4K-col slices.
Region 2 is staged host-side into rows padded to R2P columns (a fully
contiguous DRAM region degrades the SDMA engines further).
"""

import sys

for _p in ("/opt/trn_rl_repo", "/root/.axon_site/_ro/trn_rl_repo"):
    if _p not in sys.path:
        sys.path.insert(0, _p)

import numpy as np

import concourse.bass as bass
import concourse.mybir as mybir
from concourse import tile
from concourse.bass_utils import run_bass_kernel_spmd

AF = mybir.ActivationFunctionType
OP = mybir.AluOpType
F32 = mybir.dt.float32

N_CORES = 8
BUF = 1000
LN2 = 0.693147  # the reference's constant, used only inside TLS
TARGET_LOG_MEAN = float(np.log2(200.0))
TARGET_LOG_STD = 40.0 / (200.0 * LN2)
LN2_T = float(np.log(2.0))  # true ln 2

R1 = 26768
R2 = 6400
R2P = R2 + 64
SHARD = 128 * R1 + 120 * R2
assert SHARD * N_CORES == 1 << 25

# (region, width, col_offset, [load chunk cols], [act slice cols])
# stores are per act-slice except where a slice list has one entry
PLAN = [
    ("A", 3344, 0, [3344], [3344]),
    ("B", 6400, 0, [640] * 10, [3200, 3200]),
    ("A", 6688, 3344, [6688], [3344, 3344]),
    ("A", 8368, 10032, [8368], [4192, 4176]),
    ("A", 8368, 18400, [8368], [4192, 4176]),
]
assert sum(w for r, w, o, lc, ss in PLAN if r == "A") == R1
assert sum(w for r, w, o, lc, ss in PLAN if r == "B") == R2
assert all(sum(lc) == w and sum(ss) == w for r, w, o, lc, ss in PLAN)


def _legalize_sync_waits(nc, maxw=1):
    """This container's walrus accepts at most one sync-wait command per
    instruction; split extra waits into preceding same-engine NOPs."""
    n = 0
    for f in nc.m.functions:
        for bb in f.blocks:
            insts = bb.instructions
            newlist = []
            for inst in insts:
                si = inst.sync_info
                if si is not None and si.on_wait and len(si.on_wait) > maxw:
                    waits = list(si.on_wait)
                    rest = waits[-maxw:]
                    head = waits[:-maxw]
                    k = 0
                    while head:
                        chunk, head = head[:maxw], head[maxw:]
                        nop = mybir.InstNoOp(
                            name=f"{inst.name}-ws{k}",
                            sync_info=mybir.SyncInfo(
                                on_wait=list(chunk), on_update=[]
                            ),
                            engine=inst.engine,
                            bass_nofuse=True,
                        )
                        nc.register_instruction(nop)
                        newlist.append(nop)
                        k += 1
                        n += 1
                    si.on_wait.clear()
                    si.on_wait.extend(rest)
                newlist.append(inst)
            insts[:] = newlist
    return n


def _build_program(shard, tail):
    """One SPMD program: per-core two-region shard -> same-layout out, with
    the global tail [tail] + constants replicated to every core."""
    tailc = tail // 128
    assert shard == SHARD

    nc = bass.Bass()
    nbuf = 128 * R1 + 120 * R2P
    xs = nc.dram_tensor("xs", [nbuf], F32, kind="ExternalInput")
    xt = nc.dram_tensor("xt", [tail], F32, kind="ExternalInput")
    ys = nc.dram_tensor("ys", [nbuf], F32, kind="ExternalOutput")

    xv = {
        "A": xs[0 : 128 * R1].rearrange("(p c) -> p c", p=128),
        "B": xs[128 * R1 : nbuf].rearrange("(p c) -> p c", p=120),
    }
    yv = {
        "A": ys[0 : 128 * R1].rearrange("(p c) -> p c", p=128),
        "B": ys[128 * R1 : nbuf].rearrange("(p c) -> p c", p=120),
    }
    xtt = xt.rearrange("(p c) -> p c", p=128)

    sc_clamp = TARGET_LOG_STD * 1e7
    exp_bias = float(np.log(TARGET_LOG_STD * LN2_T))
    ln2sq = LN2_T * LN2_T

    with tile.TileContext(nc) as tc:
        with (
            tc.tile_pool(name="const", bufs=1) as cpool,
            tc.tile_pool(name="stat", bufs=1) as spool,
            tc.tile_pool(name="psum", bufs=1, space="PSUM") as ppool,
            tc.tile_pool(name="big", bufs=1) as bpool,
        ):
            # ---------------- phase A: stats over last BUF valid in tail
            # the tail input rides the HEAD of SP's ring: it is tiny and
            # the DVE->ACT stats chain is latency-critical.  The matmul
            # masks (strict lower-triangle + ones) are built on-chip.
            tailt = cpool.tile([128, tailc], F32)
            nc.sync.dma_start(tailt[:], xtt[:])

            I32 = mybir.dt.int32
            coli = cpool.tile([128, 128], I32)
            rowi = cpool.tile([128, 1], I32)
            nc.gpsimd.iota(coli[:], [[1, 128]], channel_multiplier=0)
            nc.gpsimd.iota(rowi[:], [[0, 1]], channel_multiplier=1)
            colf = cpool.tile([128, 128], F32)
            rowf = cpool.tile([128, 1], F32)
            nc.vector.tensor_copy(colf[:], coli[:])
            nc.vector.tensor_copy(rowf[:], rowi[:])
            ltf = cpool.tile([128, 128], F32)
            nc.vector.tensor_scalar(
                ltf[:], colf[:], rowf[:, 0:1], None, OP.is_gt
            )
            ones = cpool.tile([128, 128], F32)
            nc.vector.memset(ones[:], 1.0)

            zer = cpool.tile([128, tailc], F32)
            nc.vector.memset(zer[:], 0.0)

            mask = cpool.tile([128, tailc], F32)
            nc.vector.tensor_scalar(mask[:], tailt[:], 0.0, None, OP.is_gt)
            t1 = cpool.tile([128, tailc], F32)
            nc.vector.tensor_scalar(t1[:], tailt[:], 1.0, None, OP.max)
            lnp = cpool.tile([128, tailc], F32)
            nc.scalar.activation(lnp[:], t1[:], AF.Ln)

            # per-partition inclusive prefix count of valid
            pre = cpool.tile([128, tailc], F32)
            nc.vector.tensor_tensor_scan(
                pre[:], mask[:], zer[:], 0.0, OP.add, OP.add
            )

            # cross-partition exclusive prefix + total, via PE matmuls
            ps_rexc = ppool.tile([128, 1], F32)
            ps_vb = ppool.tile([128, 1], F32)
            last = pre[:, tailc - 1 : tailc]
            nc.tensor.matmul(ps_rexc[:], ltf[:], last)
            nc.tensor.matmul(ps_vb[:], ones[:], last)
            rexc = spool.tile([128, 1], F32)
            vb = spool.tile([128, 1], F32)
            nc.vector.tensor_copy(rexc[:], ps_rexc[:])
            nc.vector.tensor_copy(vb[:], ps_vb[:])

            # w = V - rexc - BUF ; select valid lanes with global prefix > w
            w = spool.tile([128, 1], F32)
            nc.vector.tensor_scalar(
                w[:], vb[:], rexc[:, 0:1], float(BUF), OP.subtract, OP.subtract
            )
            selg = cpool.tile([128, tailc], F32)
            nc.vector.tensor_scalar(selg[:], pre[:], w[:, 0:1], None, OP.is_gt)
            sel = cpool.tile([128, tailc], F32)
            nc.vector.tensor_tensor(sel[:], selg[:], mask[:], OP.mult)

            # one-pass moments: cnt, sum(t), sum(t^2) over selected lanes,
            # rows packed into one [128,3] tile -> single broadcast matmul
            stats = spool.tile([128, 3], F32)
            slog = cpool.tile([128, tailc], F32)
            slog2 = cpool.tile([128, tailc], F32)
            nc.vector.tensor_reduce(
                stats[:, 0:1], sel[:], mybir.AxisListType.X, OP.add
            )
            nc.vector.tensor_tensor(slog[:], sel[:], lnp[:], OP.mult)
            nc.vector.tensor_reduce(
                stats[:, 1:2], slog[:], mybir.AxisListType.X, OP.add
            )
            nc.vector.tensor_tensor(slog2[:], slog[:], lnp[:], OP.mult)
            nc.vector.tensor_reduce(
                stats[:, 2:3], slog2[:], mybir.AxisListType.X, OP.add
            )
            ps_st = ppool.tile([128, 3], F32)
            nc.tensor.matmul(ps_st[:], ones[:], stats[:, 0:3])
            bst = spool.tile([128, 3], F32)
            nc.vector.tensor_copy(bst[:], ps_st[:])
            cntb = bst[:, 0:1]
            s1b = bst[:, 1:2]
            s2b = bst[:, 2:3]

            # 1/x via exp(-ln x) on ACT: this walrus rejects the custom-DVE
            # reciprocal encoding, and x (a count >= 1) is exact enough here
            cfl = spool.tile([128, 1], F32)
            nc.vector.tensor_scalar(cfl[:], cntb, 1.0, None, OP.max)
            lncf = spool.tile([128, 1], F32)
            nc.scalar.activation(lncf[:], cfl[:], AF.Ln)
            rcp1 = spool.tile([128, 1], F32)
            nc.scalar.activation(rcp1[:], lncf[:], AF.Exp, scale=-1.0)
            meanl = spool.tile([128, 1], F32)
            nc.vector.tensor_tensor(meanl[:], s1b, rcp1[:], OP.mult)

            # unbiased variance, one-pass: (s2 - s1*mean) / max(cnt-1, 1)
            smean = spool.tile([128, 1], F32)
            nc.vector.tensor_tensor(smean[:], s1b, meanl[:], OP.mult)
            diff = spool.tile([128, 1], F32)
            nc.vector.tensor_tensor(diff[:], s2b, smean[:], OP.subtract)
            diffc = spool.tile([128, 1], F32)
            nc.vector.tensor_scalar(diffc[:], diff[:], 0.0, None, OP.max)

            den = spool.tile([128, 1], F32)
            nc.vector.tensor_scalar(
                den[:], cntb, 1.0, 1.0, OP.subtract, OP.max
            )
            lnden = spool.tile([128, 1], F32)
            nc.scalar.activation(lnden[:], den[:], AF.Ln)
            rcp2 = spool.tile([128, 1], F32)
            nc.scalar.activation(rcp2[:], lnden[:], AF.Exp, scale=-1.0)
            varl = spool.tile([128, 1], F32)
            nc.vector.tensor_tensor(varl[:], diffc[:], rcp2[:], OP.mult)

            # count<=1 -> std2 := 1  (stdL := ln2), via varL += ind*ln2^2
            ind = spool.tile([128, 1], F32)
            nc.vector.tensor_scalar(
                ind[:], cntb, 1.5, ln2sq, OP.is_lt, OP.mult
            )
            varp = spool.tile([128, 1], F32)
            nc.vector.tensor_tensor(varp[:], varl[:], ind[:], OP.add)

            # sc = TLS*ln2/sqrt(varp) = exp(-0.5*ln(varp) + ln(TLS*ln2))
            lnv = spool.tile([128, 1], F32)
            nc.scalar.activation(lnv[:], varp[:], AF.Ln)
            ebias = spool.tile([128, 1], F32)
            nc.vector.memset(ebias[:], exp_bias)
            sc_r = spool.tile([128, 1], F32)
            nc.scalar.activation(
                sc_r[:], lnv[:], AF.Exp, scale=-0.5, bias=ebias[:, 0:1]
            )
            sc = spool.tile([128, 1], F32)
            nc.vector.tensor_scalar(sc[:], sc_r[:], sc_clamp, None, OP.min)
            mb = spool.tile([128, 1], F32)
            nc.vector.tensor_tensor(mb[:], meanl[:], sc[:], OP.mult)
            bi = spool.tile([128, 1], F32)
            nc.vector.tensor_scalar(
                bi[:], mb[:], -1.0, LN2_T * TARGET_LOG_MEAN, OP.mult, OP.add
            )

            # ---------------- phase B: streamed elementwise map
            tiles = []
            for ti, (reg, w, off, lchunks, slices) in enumerate(PLAN):
                p = 128 if reg == "A" else 120
                tl = bpool.tile([p, w], F32, tag=f"t{ti}")
                s0 = 0
                for lw in lchunks:
                    nc.sync.dma_start(
                        tl[:, s0 : s0 + lw],
                        xv[reg][:, off + s0 : off + s0 + lw],
                    )
                    s0 += lw
                tiles.append(tl)
            for tl, (reg, w, off, lchunks, slices) in zip(tiles, PLAN):
                p = 128 if reg == "A" else 120
                s0 = 0
                for sw in slices:
                    cur = tl[:, s0 : s0 + sw]
                    nc.scalar.activation(cur, cur, AF.Ln)
                    nc.scalar.activation(
                        cur,
                        cur,
                        AF.Exp,
                        scale=sc[0:p, 0:1],
                        bias=bi[0:p, 0:1],
                    )
                    nc.scalar.dma_start(
                        yv[reg][:, off + s0 : off + s0 + sw], cur
                    )
                    s0 += sw

    _legalize_sync_waits(nc)
    nc.finalize()
    return nc


_cache = {}


def _get_program(shard, tail):
    key = (shard, tail)
    if key not in _cache:
        _cache[key] = _build_program(shard, tail)
    return _cache[key]


def _prep(x):
    """Build (nc, in_maps) for the full input x."""
    n = x.shape[0]
    shard = n // N_CORES
    assert n % (N_CORES * 128) == 0, f"unsupported size {n}"

    # tail window guaranteed to contain the last BUF valid elements
    tail = 16384
    while tail < n and int(np.count_nonzero(x[n - tail :] > 0.0)) < BUF:
        tail *= 2
    tail = min(tail, n)
    # phase-A SBUF tiles scale with the tail; beyond 2^16 elements they
    # would not fit alongside the streaming pool
    if tail > (1 << 16):
        # pathological density: synthesize an equivalent tail on the host
        # holding the last <=BUF valid values (stats are order-independent)
        vals = x[x > 0.0]
        kv = vals[-BUF:] if vals.size > BUF else vals
        tail = 16384
        fake = np.zeros(tail, np.float32)
        if kv.size:
            fake[-kv.size :] = kv
        xt = fake
    else:
        xt = x[n - tail :]

    nc = _get_program(shard, tail)
    in_maps = []
    for c in range(N_CORES):
        xc = np.empty(128 * R1 + 120 * R2P, np.float32)
        xc[: 128 * R1] = x[c * SHARD : c * SHARD + 128 * R1]
        x2 = xc[128 * R1 :].reshape(120, R2P)
        x2[:, :R2] = x[c * SHARD + 128 * R1 : (c + 1) * SHARD].reshape(
            120, R2
        )
        x2[:, R2:] = 0.0
        in_maps.append({"xs": xc, "xt": xt})
    return nc, in_maps


def _assemble(res):
    out = np.empty(N_CORES * SHARD, np.float32)
    for c in range(N_CORES):
        base = c * SHARD
        yc = res.results[c]["ys"]
        out[base : base + 128 * R1] = yc[: 128 * R1]
        out[base + 128 * R1 : base + SHARD] = (
            yc[128 * R1 :].reshape(120, R2P)[:, :R2].reshape(-1)
        )
    return out


def kernel(pitch_values):
    x = np.ascontiguousarray(np.asarray(pitch_values, dtype=np.float32))
    nc, in_maps = _prep(x)
    res = run_bass_kernel_spmd(nc, in_maps, core_ids=list(range(N_CORES)))
    return _assemble(res)
